# revision 29
# baseline (speedup 1.0000x reference)
"""Trainium2 Bass kernel for sliding-window GQA attention (nn_Attention_62294205661445).

Sharding: 8 cores = 4 batches x 2 head-groups. Each core computes one batch's
attention for 8 q-heads / 2 kv-heads and a partial output projection over its
512 columns of the H*HD dim; the host sums the two partials per batch.

Per-core pipeline (all matmul operands bf16, 1 cyc/row; fp32 accumulate):
  phase 1: QKV projection (xT tiles, contraction d=1152), RoPE on [s,e] layout
           with stride-2 APs, PE-transpose q/k to [e,s], cache writeout.
  phase 2: transposed scores ST[j,i] = K·Q^T per 512-col supertile, additive
           sliding-window mask patterns, exp on ScalarE (no max subtraction:
           |scores*0.125| < ~6 so fp32 exp is safe), PV via ones-row-augmented
           V (row 64 of the PV psum accumulates the softmax denominator),
           reciprocal + gpsimd partition_broadcast, normalize, then WO.
"""
import sys
import numpy as np
import ml_dtypes

for p in ('/opt/trn_rl_repo',):
    if p not in sys.path:
        sys.path.insert(0, p)

import concourse.bass as bass
import concourse.tile as tile
from concourse import bacc
from concourse import mybir
from concourse.bass_utils import run_bass_kernel_spmd
from concourse.masks import make_identity

B, S, D = 4, 2048, 1152
H, KVH, HD = 16, 4, 64
SW = 1792
NT = S // 128          # 16 s-tiles
NG = NT // 4           # 4 supertiles (512 queries each)
DC = D // 128          # 9 contraction chunks
F32 = mybir.dt.float32
F32R = mybir.dt.float32r
BF16 = mybir.dt.bfloat16

_CACHED = {}


def r(ap):
    """View an AP as float32r (fast fp32 matmul mode)."""
    return ap.bitcast(F32R)


def build_graph():
    nc = bacc.Bacc()
    xT = nc.declare_dram_parameter("xT", [D, S], BF16, isOutput=False)
    wqT = nc.declare_dram_parameter("wqT", [D, 512], BF16, isOutput=False)
    wkvT = nc.declare_dram_parameter("wkvT", [D, 256], BF16, isOutput=False)
    woT = nc.declare_dram_parameter("woT", [512, D], BF16, isOutput=False)
    cosr = nc.declare_dram_parameter("cosrep", [S, 256], BF16, isOutput=False)
    sinr = nc.declare_dram_parameter("sinrep", [S, 256], BF16, isOutput=False)
    mdiag = nc.declare_dram_parameter("mdiagT", [128, 512], F32, isOutput=False)
    mtail = nc.declare_dram_parameter("mtailT", [128, 512], F32, isOutput=False)
    out_e = nc.declare_dram_parameter("out", [S, D], F32, isOutput=True)
    ck_e = nc.declare_dram_parameter("ck", [SW, 2, HD], F32, isOutput=True)
    cv_e = nc.declare_dram_parameter("cv", [SW, 2, HD], F32, isOutput=True)

    with tile.TileContext(nc) as tc:
        with (
            tc.tile_pool(name="const", bufs=1) as constp,
            tc.tile_pool(name="w", bufs=1) as wp,
            tc.tile_pool(name="persist", bufs=1) as pers,
            tc.tile_pool(name="xt", bufs=5) as xtp,
            tc.tile_pool(name="rope", bufs=3) as ropep,
            tc.tile_pool(name="ptile", bufs=14) as ppool,
            tc.tile_pool(name="outb", bufs=2) as outp,
            tc.tile_pool(name="small", bufs=2) as smallp,
            tc.tile_pool(name="attn", bufs=2) as atp,
            tc.tile_pool(name="psA", bufs=2, space="PSUM") as psA,
            tc.tile_pool(name="psB", bufs=2, space="PSUM") as psB,
            tc.tile_pool(name="psC", bufs=2, space="PSUM") as psC,
        ):
            # x tiles + per-chunk weights first, tables after
            xts = {}

            def load_xt(t):
                xt = xtp.tile([128, DC, 128], BF16, tag="xt")
                nc.sync.dma_start(
                    xt[:], xT[:, t*128:(t+1)*128].rearrange(
                        "(c p) s -> p c s", p=128))
                xts[t] = xt

            load_xt(0)
            wq_sb = wp.tile([128, DC, 512], BF16, tag="wq")
            wkv_sb = wp.tile([128, DC, 256], BF16, tag="wkv")
            for c in range(DC):
                eng = nc.scalar if c % 2 else nc.sync
                eng.dma_start(wq_sb[:, c, :], wqT[c*128:(c+1)*128, :])
                eng.dma_start(wkv_sb[:, c, :], wkvT[c*128:(c+1)*128, :])
            load_xt(1)
            load_xt(2)
            cos_sb = wp.tile([128, NT, 256], BF16, tag="cos")
            nc.scalar.dma_start(
                cos_sb[:], cosr[:].rearrange("(t p) m -> p t m", p=128))
            sin_sb = wp.tile([128, NT, 256], BF16, tag="sin")
            nc.sync.dma_start(
                sin_sb[:], sinr[:].rearrange("(t p) m -> p t m", p=128))
            md_sb = constp.tile([128, 512], F32, tag="md")
            nc.scalar.dma_start(md_sb[:], mdiag[:])
            mt_sb = constp.tile([128, 512], F32, tag="mt")
            nc.scalar.dma_start(mt_sb[:], mtail[:])
            wo_sb = wp.tile([128, 4, D], BF16, tag="wo")
            nc.gpsimd.dma_start(
                wo_sb[:], woT[:].rearrange("(c p) e -> p c e", p=128))
            ident = constp.tile([128, 128], BF16)
            make_identity(nc, ident[:])

            # persistent activations, sharded per supertile so attention
            # for gi only waits on phase-1 tiles 4gi..4gi+3 (whole-tile dep
            # granularity would otherwise serialize the phases)
            qT_q = [pers.tile([128, 4, 512], BF16, tag=f"qT{g}",
                              name=f"qT{g}") for g in range(NG)]
            kT_q = [pers.tile([128, 512], BF16, tag=f"kT{g}",
                              name=f"kT{g}") for g in range(NG)]
            vaug_q = [pers.tile([128, 4, 130], BF16, tag=f"va{g}",
                                name=f"va{g}") for g in range(NG)]
            for g in range(NG):
                nc.gpsimd.memset(vaug_q[g][:, :, 64::65], 1.0)

            # ---------------- phase 1: projections + RoPE + transposes -------
            stash = {}
            for t in range(NT):
                if t + 3 < NT:
                    load_xt(t + 3)
                xt = xts.pop(t)
                q_ps = psC.tile([128, 512], F32, tag="C")
                kv_ps = psB.tile([128, 256], F32, tag="B")
                for c in range(DC):
                    nc.tensor.matmul(q_ps[:], xt[:, c, :], wq_sb[:, c, :],
                                     start=(c == 0), stop=(c == DC-1))
                for c in range(DC):
                    nc.tensor.matmul(kv_ps[:], xt[:, c, :], wkv_sb[:, c, :],
                                     start=(c == 0), stop=(c == DC-1))

                # RoPE: q
                qro = ropep.tile([128, 512], BF16, tag="qro")
                C = cos_sb[:, t, :]
                Sn = sin_sb[:, t, :]
                ta = ropep.tile([128, 256], F32, tag="ta")
                tb = ropep.tile([128, 256], F32, tag="tb")
                a, b = q_ps[:, 0:512:2], q_ps[:, 1:512:2]
                nc.vector.tensor_mul(ta[:], a, C)
                nc.vector.tensor_mul(tb[:], b, Sn)
                nc.vector.tensor_sub(qro[:, 0:512:2], ta[:], tb[:])
                nc.vector.tensor_mul(ta[:], a, Sn)
                nc.vector.tensor_mul(tb[:], b, C)
                nc.vector.tensor_add(qro[:, 1:512:2], ta[:], tb[:])
                # RoPE: k
                kro = ropep.tile([128, 128], BF16, tag="kro")
                ka, kb = kv_ps[:, 0:128:2], kv_ps[:, 1:128:2]
                C2, S2 = cos_sb[:, t, 0:64], sin_sb[:, t, 0:64]
                ka1 = ropep.tile([128, 64], F32, tag="ka1")
                kb1 = ropep.tile([128, 64], F32, tag="kb1")
                nc.vector.tensor_mul(ka1[:], ka, C2)
                nc.vector.tensor_mul(kb1[:], kb, S2)
                nc.vector.tensor_sub(kro[:, 0:128:2], ka1[:], kb1[:])
                nc.vector.tensor_mul(ka1[:], ka, S2)
                nc.vector.tensor_mul(kb1[:], kb, C2)
                nc.vector.tensor_add(kro[:, 1:128:2], ka1[:], kb1[:])

                # v into augmented layout (ScalarE copy, psum->sbuf)
                nc.scalar.copy(
                    vaug_q[t//4][:, t % 4, :].rearrange(
                        "p (h x) -> p h x", h=2)[:, :, 0:64],
                    kv_ps[:, 128:256].rearrange("p (h x) -> p h x", h=2))

                # cache writeout for s >= 256
                if t >= 2:
                    s0 = t * 128
                    slot = s0 - SW if s0 >= SW else s0
                    nc.gpsimd.dma_start(
                        ck_e[slot:slot+128],
                        kro[:].rearrange("p (h x) -> p h x", h=2))
                    nc.gpsimd.dma_start(
                        cv_e[slot:slot+128],
                        vaug_q[t//4][:, t % 4, :].rearrange(
                            "p (h x) -> p h x", h=2)[:, :, 0:64])

                # transposes to [e, s] layout (PE transpose, bf16)
                for cc in range(4):
                    tr = psC.tile([128, 128], BF16, tag="C")
                    nc.tensor.transpose(tr[:], qro[:, cc*128:(cc+1)*128], ident[:])
                    nc.vector.tensor_copy(
                        qT_q[t//4][:, cc, (t % 4)*128:(t % 4 + 1)*128], tr[:])
                trk = psC.tile([128, 128], BF16, tag="C")
                nc.tensor.transpose(trk[:], kro[:], ident[:])
                nc.vector.tensor_copy(
                    kT_q[t//4][:, (t % 4)*128:(t % 4 + 1)*128], trk[:])

            # ---------------- phase 2: attention + WO ----------------------
            # GQA-packed: one ST matmul computes scores of ALL 4 q-heads of a
            # kv group for one (q-tile, key-block): rhs = qT[64, 4 heads, 128]
            # -> st[j, 4*128]. One PV matmul serves all 4 heads (shared V).
            for gi in range(NG):
                at_sb = atp.tile([128, 4, 512], BF16, tag="at")
                for ql in range(4):
                    qi = gi * 4 + ql
                    for h2 in range(2):
                        kjs = list(range(max(0, qi - 14), qi + 1))
                        ptiles = {}
                        # ST + masks + exp, two kj per psum tile
                        for idx in range(0, len(kjs), 2):
                            pair = kjs[idx:idx+2]
                            st = psA.tile([128, 1024], F32, tag="A")
                            p = ppool.tile([128, 1024], BF16, tag="p")
                            masked = []
                            for half, kj in enumerate(pair):
                                o = half * 512
                                nc.tensor.matmul(
                                    st[:, o:o+512],
                                    kT_q[kj//4][h2*64:(h2+1)*64,
                                                (kj % 4)*128:(kj % 4 + 1)*128],
                                    qT_q[qi//4][h2*64:h2*64+64, 0:4,
                                                (qi % 4)*128:(qi % 4 + 1)*128],
                                    start=True, stop=True)
                                if kj == qi or kj == qi - 14:
                                    masked.append((kj, o))
                                ptiles[kj] = (p, o)
                            nc.scalar.activation(
                                p[:, 0:512*len(pair)], st[:, 0:512*len(pair)],
                                mybir.ActivationFunctionType.Exp, scale=0.125)
                            # sliding-window masking: zero the invalid
                            # triangle of P on the idle GpSimd engine (the
                            # ones-row PV sums P after this, so softmax
                            # denominators stay exact)
                            for kj, o in masked:
                                sl = p[:, o:o+512].rearrange(
                                    "p (h x) -> p h x", h=4)
                                if kj == qi:
                                    # keep j <= i  (col - row >= 0)
                                    nc.gpsimd.affine_select(
                                        out=sl, in_=sl,
                                        compare_op=mybir.AluOpType.is_ge,
                                        fill=0.0, base=0,
                                        pattern=[[0, 4], [1, 128]],
                                        channel_multiplier=-1)
                                else:
                                    # tail block: keep i - j < SW (row > col)
                                    nc.gpsimd.affine_select(
                                        out=sl, in_=sl,
                                        compare_op=mybir.AluOpType.is_gt,
                                        fill=0.0, base=0,
                                        pattern=[[0, 4], [-1, 128]],
                                        channel_multiplier=1)
                        # PV accumulation over kj (shared V for the 4 heads,
                        # ones row 64 collects the softmax denominators)
                        pv = psB.tile([65, 512], F32, tag="B")
                        for kj in kjs:
                            p_, o_ = ptiles[kj]
                            nc.tensor.matmul(
                                pv[:],
                                vaug_q[kj//4][:, kj % 4, h2*65:(h2+1)*65],
                                p_[:, o_:o_+512],
                                start=(kj == kjs[0]), stop=(kj == kjs[-1]))
                        rs = smallp.tile([1, 512], F32, tag="rs")
                        nc.vector.tensor_copy(rs[:], pv[64:65, :])
                        rr = smallp.tile([1, 512], F32, tag="rr")
                        nc.vector.reciprocal_approx_fast(rr[:], rs[:])
                        Rb = smallp.tile([64, 512], F32, tag="Rb")
                        nc.gpsimd.partition_broadcast(Rb[:], rr[:])
                        nc.vector.tensor_mul(
                            at_sb[h2*64:h2*64+64, :, ql*128:(ql+1)*128],
                            pv[0:64, :].rearrange("p (h x) -> p h x", h=4),
                            Rb[:].rearrange("p (h x) -> p h x", h=4))
                # WO: out[s, d] partial for this supertile
                for ss in range(4):
                    osb = outp.tile([128, D], F32, tag="osb")
                    for nn in range(3):
                        wo_ps = psC.tile([128, 384], F32, tag="C")
                        for c in range(4):
                            nc.tensor.matmul(
                                wo_ps[:],
                                at_sb[:, c, ss*128:(ss+1)*128],
                                wo_sb[:, c, nn*384:(nn+1)*384],
                                start=(c == 0), stop=(c == 3))
                        nc.vector.tensor_copy(osb[:, nn*384:(nn+1)*384], wo_ps[:])
                    nc.sync.dma_start(
                        out_e[gi*512+ss*128:gi*512+(ss+1)*128, :], osb[:])
    nc.finalize()
    return nc


def _prep_inputs(x, wq, wk, wv, wo, freqs_cos, freqs_sin, mask):
    cosrep = np.ascontiguousarray(
        np.tile(np.asarray(freqs_cos), (1, 8))).astype(ml_dtypes.bfloat16)
    sinrep = np.ascontiguousarray(
        np.tile(np.asarray(freqs_sin), (1, 8))).astype(ml_dtypes.bfloat16)
    mdiagT = np.ascontiguousarray(
        np.tile(np.asarray(mask)[0:128, 0:128].T, (1, 4)), dtype=np.float32)
    mtailT = np.ascontiguousarray(
        np.tile(np.asarray(mask)[1792:1920, 0:128].T, (1, 4)), dtype=np.float32)
    # head-block permutation: etile c holds local heads (c, c+4) so that each
    # q head's partition half matches its kv head's half in kT
    hperm = np.array([0, 4, 1, 5, 2, 6, 3, 7])
    eperm = (hperm[:, None] * 64 + np.arange(64)[None, :]).reshape(-1)
    in_maps = []
    for b in range(B):
        for g in range(2):
            wq_g = np.asarray(wq)[512*g:512*(g+1)][eperm]      # [512, 1152]
            wo_g = np.asarray(wo)[:, 512*g:512*(g+1)][:, eperm]  # [1152, 512]
            m = {
                "xT": np.ascontiguousarray(np.asarray(x)[b].T).astype(ml_dtypes.bfloat16),
                "wqT": np.ascontiguousarray(wq_g.T).astype(ml_dtypes.bfloat16),
                "wkvT": np.ascontiguousarray(np.concatenate(
                    [np.asarray(wk)[128*g:128*(g+1)],
                     np.asarray(wv)[128*g:128*(g+1)]], 0).T).astype(
                         ml_dtypes.bfloat16),
                "woT": np.ascontiguousarray(wo_g.T).astype(ml_dtypes.bfloat16),
                "cosrep": cosrep, "sinrep": sinrep,
                "mdiagT": mdiagT, "mtailT": mtailT,
            }
            in_maps.append(m)
    return in_maps


def kernel(x, wq, wk, wv, wo, freqs_cos, freqs_sin, mask, cache_k, cache_v,
           positions, _trace=False):
    if 'nc' not in _CACHED:
        _CACHED['nc'] = build_graph()
    nc = _CACHED['nc']
    in_maps = _prep_inputs(x, wq, wk, wv, wo, freqs_cos, freqs_sin, mask)
    res = run_bass_kernel_spmd(nc, in_maps, core_ids=list(range(8)),
                               trace=_trace)
    outs = res.results
    out = np.zeros((B, S, D), np.float32)
    ck = np.zeros((B, SW, KVH, HD), np.float32)
    cv = np.zeros((B, SW, KVH, HD), np.float32)
    for b in range(B):
        for g in range(2):
            c = b*2 + g
            out[b] += outs[c]["out"]
            ck[b, :, 2*g:2*g+2] = outs[c]["ck"]
            cv[b, :, 2*g:2*g+2] = outs[c]["cv"]
    if _trace:
        return (out, ck, cv), res
    return out, ck, cv


# revision 30
# speedup vs baseline: 1.0964x; 1.0964x over previous
"""Trainium2 Bass kernel for sliding-window GQA attention (nn_Attention_62294205661445).

Sharding: 8 cores = 4 batches x 2 head-groups. Each core computes one batch's
attention for 8 q-heads / 2 kv-heads and a partial output projection over its
512 columns of the H*HD dim; the host sums the two partials per batch.

Per-core pipeline (all matmul operands bf16, 1 cyc/row; fp32 accumulate):
  phase 1: QKV projection (xT tiles, contraction d=1152), RoPE on [s,e] layout
           with stride-2 APs, PE-transpose q/k to [e,s], cache writeout.
  phase 2: transposed scores ST[j,i] = K·Q^T per 512-col supertile, additive
           sliding-window mask patterns, exp on ScalarE (no max subtraction:
           |scores*0.125| < ~6 so fp32 exp is safe), PV via ones-row-augmented
           V (row 64 of the PV psum accumulates the softmax denominator),
           reciprocal + gpsimd partition_broadcast, normalize, then WO.
"""
import sys
import numpy as np
import ml_dtypes

for p in ('/opt/trn_rl_repo',):
    if p not in sys.path:
        sys.path.insert(0, p)

import concourse.bass as bass
import concourse.tile as tile
from concourse import bacc
from concourse import mybir
from concourse.bass_utils import run_bass_kernel_spmd
from concourse.masks import make_identity

B, S, D = 4, 2048, 1152
H, KVH, HD = 16, 4, 64
SW = 1792
NT = S // 128          # 16 s-tiles
NG = NT // 4           # 4 supertiles (512 queries each)
DC = D // 128          # 9 contraction chunks
F32 = mybir.dt.float32
F32R = mybir.dt.float32r
BF16 = mybir.dt.bfloat16

_CACHED = {}


def r(ap):
    """View an AP as float32r (fast fp32 matmul mode)."""
    return ap.bitcast(F32R)


def build_graph():
    nc = bacc.Bacc()
    xT = nc.declare_dram_parameter("xT", [D, S], BF16, isOutput=False)
    wqT = nc.declare_dram_parameter("wqT", [D, 512], BF16, isOutput=False)
    wkvT = nc.declare_dram_parameter("wkvT", [D, 256], BF16, isOutput=False)
    woT = nc.declare_dram_parameter("woT", [512, D], BF16, isOutput=False)
    cosr = nc.declare_dram_parameter("cosrep", [S, 256], BF16, isOutput=False)
    sinr = nc.declare_dram_parameter("sinrep", [S, 256], BF16, isOutput=False)
    mdiag = nc.declare_dram_parameter("mdiagT", [128, 512], F32, isOutput=False)
    mtail = nc.declare_dram_parameter("mtailT", [128, 512], F32, isOutput=False)
    out_e = nc.declare_dram_parameter("out", [S, D], F32, isOutput=True)
    ck_e = nc.declare_dram_parameter("ck", [SW, 2, HD], F32, isOutput=True)
    cv_e = nc.declare_dram_parameter("cv", [SW, 2, HD], F32, isOutput=True)

    with tile.TileContext(nc) as tc:
        with (
            tc.tile_pool(name="const", bufs=1) as constp,
            tc.tile_pool(name="w", bufs=1) as wp,
            tc.tile_pool(name="persist", bufs=1) as pers,
            tc.tile_pool(name="xt", bufs=5) as xtp,
            tc.tile_pool(name="rope", bufs=3) as ropep,
            tc.tile_pool(name="ptile", bufs=14) as ppool,
            tc.tile_pool(name="outb", bufs=2) as outp,
            tc.tile_pool(name="small", bufs=2) as smallp,
            tc.tile_pool(name="attn", bufs=2) as atp,
            tc.tile_pool(name="psA", bufs=2, space="PSUM") as psA,
            tc.tile_pool(name="psB", bufs=2, space="PSUM") as psB,
            tc.tile_pool(name="psC", bufs=2, space="PSUM") as psC,
        ):
            # x tiles + per-chunk weights first, tables after
            xts = {}

            def load_xt(t):
                xt = xtp.tile([128, DC, 128], BF16, tag="xt")
                nc.sync.dma_start(
                    xt[:], xT[:, t*128:(t+1)*128].rearrange(
                        "(c p) s -> p c s", p=128))
                xts[t] = xt

            load_xt(0)
            wq_sb = wp.tile([128, DC, 512], BF16, tag="wq")
            wkv_sb = wp.tile([128, DC, 256], BF16, tag="wkv")
            for c in range(DC):
                eng = nc.scalar if c % 2 else nc.sync
                eng.dma_start(wq_sb[:, c, :], wqT[c*128:(c+1)*128, :])
                eng.dma_start(wkv_sb[:, c, :], wkvT[c*128:(c+1)*128, :])
            load_xt(1)
            load_xt(2)
            cos_sb = wp.tile([128, NT, 256], BF16, tag="cos")
            nc.scalar.dma_start(
                cos_sb[:], cosr[:].rearrange("(t p) m -> p t m", p=128))
            sin_sb = wp.tile([128, NT, 256], BF16, tag="sin")
            nc.sync.dma_start(
                sin_sb[:], sinr[:].rearrange("(t p) m -> p t m", p=128))
            md_sb = constp.tile([128, 512], F32, tag="md")
            nc.scalar.dma_start(md_sb[:], mdiag[:])
            mt_sb = constp.tile([128, 512], F32, tag="mt")
            nc.scalar.dma_start(mt_sb[:], mtail[:])
            wo_sb = wp.tile([128, 4, D], BF16, tag="wo")
            nc.gpsimd.dma_start(
                wo_sb[:], woT[:].rearrange("(c p) e -> p c e", p=128))
            ident = constp.tile([128, 128], BF16)
            make_identity(nc, ident[:])

            # persistent activations, sharded per supertile so attention
            # for gi only waits on phase-1 tiles 4gi..4gi+3 (whole-tile dep
            # granularity would otherwise serialize the phases)
            qT_q = [pers.tile([128, 4, 512], BF16, tag=f"qT{g}",
                              name=f"qT{g}") for g in range(NG)]
            kT_q = [pers.tile([128, 512], BF16, tag=f"kT{g}",
                              name=f"kT{g}") for g in range(NG)]
            vaug_q = [pers.tile([128, 4, 130], BF16, tag=f"va{g}",
                                name=f"va{g}") for g in range(NG)]
            for g in range(NG):
                nc.gpsimd.memset(vaug_q[g][:, :, 64::65], 1.0)

            # ---------------- phase 1: projections + RoPE + transposes -------
            stash = {}
            for t in range(NT):
                if t + 3 < NT:
                    load_xt(t + 3)
                xt = xts.pop(t)
                q_ps = psA.tile([128, 512], F32, tag="A")
                kv_ps = psB.tile([128, 256], F32, tag="B")
                for c in range(DC):
                    nc.tensor.matmul(q_ps[:], xt[:, c, :], wq_sb[:, c, :],
                                     start=(c == 0), stop=(c == DC-1))
                for c in range(DC):
                    nc.tensor.matmul(kv_ps[:], xt[:, c, :], wkv_sb[:, c, :],
                                     start=(c == 0), stop=(c == DC-1))

                # RoPE: q
                qro = ropep.tile([128, 512], BF16, tag="qro")
                C = cos_sb[:, t, :]
                Sn = sin_sb[:, t, :]
                ta = ropep.tile([128, 256], F32, tag="ta")
                tb = ropep.tile([128, 256], F32, tag="tb")
                a, b = q_ps[:, 0:512:2], q_ps[:, 1:512:2]
                nc.vector.tensor_mul(ta[:], a, C)
                nc.vector.tensor_mul(tb[:], b, Sn)
                nc.vector.tensor_sub(qro[:, 0:512:2], ta[:], tb[:])
                nc.vector.tensor_mul(ta[:], a, Sn)
                nc.vector.tensor_mul(tb[:], b, C)
                nc.vector.tensor_add(qro[:, 1:512:2], ta[:], tb[:])
                # RoPE: k
                kro = ropep.tile([128, 128], BF16, tag="kro")
                ka, kb = kv_ps[:, 0:128:2], kv_ps[:, 1:128:2]
                C2, S2 = cos_sb[:, t, 0:64], sin_sb[:, t, 0:64]
                ka1 = ropep.tile([128, 64], F32, tag="ka1")
                kb1 = ropep.tile([128, 64], F32, tag="kb1")
                nc.vector.tensor_mul(ka1[:], ka, C2)
                nc.vector.tensor_mul(kb1[:], kb, S2)
                nc.vector.tensor_sub(kro[:, 0:128:2], ka1[:], kb1[:])
                nc.vector.tensor_mul(ka1[:], ka, S2)
                nc.vector.tensor_mul(kb1[:], kb, C2)
                nc.vector.tensor_add(kro[:, 1:128:2], ka1[:], kb1[:])

                # v into augmented layout (ScalarE copy, psum->sbuf)
                nc.scalar.copy(
                    vaug_q[t//4][:, t % 4, :].rearrange(
                        "p (h x) -> p h x", h=2)[:, :, 0:64],
                    kv_ps[:, 128:256].rearrange("p (h x) -> p h x", h=2))

                # cache writeout for s >= 256
                if t >= 2:
                    s0 = t * 128
                    slot = s0 - SW if s0 >= SW else s0
                    nc.gpsimd.dma_start(
                        ck_e[slot:slot+128],
                        kro[:].rearrange("p (h x) -> p h x", h=2))
                    nc.gpsimd.dma_start(
                        cv_e[slot:slot+128],
                        vaug_q[t//4][:, t % 4, :].rearrange(
                            "p (h x) -> p h x", h=2)[:, :, 0:64])

                # transposes to [e, s] layout (PE transpose, bf16)
                for cc in range(4):
                    tr = psC.tile([128, 128], BF16, tag="C")
                    nc.tensor.transpose(tr[:], qro[:, cc*128:(cc+1)*128], ident[:])
                    nc.vector.tensor_copy(
                        qT_q[t//4][:, cc, (t % 4)*128:(t % 4 + 1)*128], tr[:])
                trk = psC.tile([128, 128], BF16, tag="C")
                nc.tensor.transpose(trk[:], kro[:], ident[:])
                nc.vector.tensor_copy(
                    kT_q[t//4][:, (t % 4)*128:(t % 4 + 1)*128], trk[:])

            # ---------------- phase 2: attention + WO ----------------------
            # GQA-packed: one ST matmul computes scores of ALL 4 q-heads of a
            # kv group for one (q-tile, key-block): rhs = qT[64, 4 heads, 128]
            # -> st[j, 4*128]. One PV matmul serves all 4 heads (shared V).
            for gi in range(NG):
                at_sb = atp.tile([128, 4, 512], BF16, tag="at")
                for ql in range(4):
                    qi = gi * 4 + ql
                    for h2 in range(2):
                        kjs = list(range(max(0, qi - 14), qi + 1))
                        ptiles = {}
                        # ST + masks + exp, two kj per psum tile
                        for idx in range(0, len(kjs), 2):
                            pair = kjs[idx:idx+2]
                            st = psA.tile([128, 1024], F32, tag="A")
                            p = ppool.tile([128, 1024], BF16, tag="p")
                            masked = []
                            for half, kj in enumerate(pair):
                                o = half * 512
                                nc.tensor.matmul(
                                    st[:, o:o+512],
                                    kT_q[kj//4][h2*64:(h2+1)*64,
                                                (kj % 4)*128:(kj % 4 + 1)*128],
                                    qT_q[qi//4][h2*64:h2*64+64, 0:4,
                                                (qi % 4)*128:(qi % 4 + 1)*128],
                                    start=True, stop=True)
                                if kj == qi or kj == qi - 14:
                                    masked.append((kj, o))
                                ptiles[kj] = (p, o)
                            nc.scalar.activation(
                                p[:, 0:512*len(pair)], st[:, 0:512*len(pair)],
                                mybir.ActivationFunctionType.Exp, scale=0.125)
                            # sliding-window masking: zero the invalid
                            # triangle of P on the idle GpSimd engine (the
                            # ones-row PV sums P after this, so softmax
                            # denominators stay exact)
                            for kj, o in masked:
                                sl = p[:, o:o+512].rearrange(
                                    "p (h x) -> p h x", h=4)
                                if kj == qi:
                                    # keep j <= i  (col - row >= 0)
                                    nc.gpsimd.affine_select(
                                        out=sl, in_=sl,
                                        compare_op=mybir.AluOpType.is_ge,
                                        fill=0.0, base=0,
                                        pattern=[[0, 4], [1, 128]],
                                        channel_multiplier=-1)
                                else:
                                    # tail block: keep i - j < SW (row > col)
                                    nc.gpsimd.affine_select(
                                        out=sl, in_=sl,
                                        compare_op=mybir.AluOpType.is_gt,
                                        fill=0.0, base=0,
                                        pattern=[[0, 4], [-1, 128]],
                                        channel_multiplier=1)
                        # PV accumulation over kj (shared V for the 4 heads,
                        # ones row 64 collects the softmax denominators)
                        pv = psB.tile([65, 512], F32, tag="B")
                        for kj in kjs:
                            p_, o_ = ptiles[kj]
                            nc.tensor.matmul(
                                pv[:],
                                vaug_q[kj//4][:, kj % 4, h2*65:(h2+1)*65],
                                p_[:, o_:o_+512],
                                start=(kj == kjs[0]), stop=(kj == kjs[-1]))
                        rs = smallp.tile([1, 512], F32, tag="rs")
                        nc.vector.tensor_copy(rs[:], pv[64:65, :])
                        rr = smallp.tile([1, 512], F32, tag="rr")
                        nc.vector.reciprocal_approx_fast(rr[:], rs[:])
                        Rb = smallp.tile([64, 512], F32, tag="Rb")
                        nc.gpsimd.partition_broadcast(Rb[:], rr[:])
                        nc.vector.tensor_mul(
                            at_sb[h2*64:h2*64+64, :, ql*128:(ql+1)*128],
                            pv[0:64, :].rearrange("p (h x) -> p h x", h=4),
                            Rb[:].rearrange("p (h x) -> p h x", h=4))
                # WO: out[s, d] partial for this supertile
                for ss in range(4):
                    osb = outp.tile([128, D], F32, tag="osb")
                    for nn in range(3):
                        wo_ps = psC.tile([128, 384], F32, tag="C")
                        for c in range(4):
                            nc.tensor.matmul(
                                wo_ps[:],
                                at_sb[:, c, ss*128:(ss+1)*128],
                                wo_sb[:, c, nn*384:(nn+1)*384],
                                start=(c == 0), stop=(c == 3))
                        nc.vector.tensor_copy(osb[:, nn*384:(nn+1)*384], wo_ps[:])
                    nc.sync.dma_start(
                        out_e[gi*512+ss*128:gi*512+(ss+1)*128, :], osb[:])
    nc.finalize()
    return nc


def _prep_inputs(x, wq, wk, wv, wo, freqs_cos, freqs_sin, mask):
    cosrep = np.ascontiguousarray(
        np.tile(np.asarray(freqs_cos), (1, 8))).astype(ml_dtypes.bfloat16)
    sinrep = np.ascontiguousarray(
        np.tile(np.asarray(freqs_sin), (1, 8))).astype(ml_dtypes.bfloat16)
    mdiagT = np.ascontiguousarray(
        np.tile(np.asarray(mask)[0:128, 0:128].T, (1, 4)), dtype=np.float32)
    mtailT = np.ascontiguousarray(
        np.tile(np.asarray(mask)[1792:1920, 0:128].T, (1, 4)), dtype=np.float32)
    # head-block permutation: etile c holds local heads (c, c+4) so that each
    # q head's partition half matches its kv head's half in kT
    hperm = np.array([0, 4, 1, 5, 2, 6, 3, 7])
    eperm = (hperm[:, None] * 64 + np.arange(64)[None, :]).reshape(-1)
    in_maps = []
    for b in range(B):
        for g in range(2):
            wq_g = np.asarray(wq)[512*g:512*(g+1)][eperm]      # [512, 1152]
            wo_g = np.asarray(wo)[:, 512*g:512*(g+1)][:, eperm]  # [1152, 512]
            m = {
                "xT": np.ascontiguousarray(np.asarray(x)[b].T).astype(ml_dtypes.bfloat16),
                "wqT": np.ascontiguousarray(wq_g.T).astype(ml_dtypes.bfloat16),
                "wkvT": np.ascontiguousarray(np.concatenate(
                    [np.asarray(wk)[128*g:128*(g+1)],
                     np.asarray(wv)[128*g:128*(g+1)]], 0).T).astype(
                         ml_dtypes.bfloat16),
                "woT": np.ascontiguousarray(wo_g.T).astype(ml_dtypes.bfloat16),
                "cosrep": cosrep, "sinrep": sinrep,
                "mdiagT": mdiagT, "mtailT": mtailT,
            }
            in_maps.append(m)
    return in_maps


def kernel(x, wq, wk, wv, wo, freqs_cos, freqs_sin, mask, cache_k, cache_v,
           positions, _trace=False):
    if 'nc' not in _CACHED:
        _CACHED['nc'] = build_graph()
    nc = _CACHED['nc']
    in_maps = _prep_inputs(x, wq, wk, wv, wo, freqs_cos, freqs_sin, mask)
    res = run_bass_kernel_spmd(nc, in_maps, core_ids=list(range(8)),
                               trace=_trace)
    outs = res.results
    out = np.zeros((B, S, D), np.float32)
    ck = np.zeros((B, SW, KVH, HD), np.float32)
    cv = np.zeros((B, SW, KVH, HD), np.float32)
    for b in range(B):
        for g in range(2):
            c = b*2 + g
            out[b] += outs[c]["out"]
            ck[b, :, 2*g:2*g+2] = outs[c]["ck"]
            cv[b, :, 2*g:2*g+2] = outs[c]["cv"]
    if _trace:
        return (out, ck, cv), res
    return out, ck, cv


# revision 31
# speedup vs baseline: 1.1103x; 1.0127x over previous
"""Trainium2 Bass kernel for sliding-window GQA attention (nn_Attention_62294205661445).

Sharding: 8 cores = 4 batches x 2 head-groups. Each core computes one batch's
attention for 8 q-heads / 2 kv-heads and a partial output projection over its
512 columns of the H*HD dim; the host sums the two partials per batch.

Per-core pipeline (all matmul operands bf16, 1 cyc/row; fp32 accumulate):
  phase 1: QKV projection (xT tiles, contraction d=1152), RoPE on [s,e] layout
           with stride-2 APs, PE-transpose q/k to [e,s], cache writeout.
  phase 2: transposed scores ST[j,i] = K·Q^T per 512-col supertile, additive
           sliding-window mask patterns, exp on ScalarE (no max subtraction:
           |scores*0.125| < ~6 so fp32 exp is safe), PV via ones-row-augmented
           V (row 64 of the PV psum accumulates the softmax denominator),
           reciprocal + gpsimd partition_broadcast, normalize, then WO.
"""
import sys
import numpy as np
import ml_dtypes

for p in ('/opt/trn_rl_repo',):
    if p not in sys.path:
        sys.path.insert(0, p)

import concourse.bass as bass
import concourse.tile as tile
from concourse import bacc
from concourse import mybir
from concourse.bass_utils import run_bass_kernel_spmd
from concourse.masks import make_identity

B, S, D = 4, 2048, 1152
H, KVH, HD = 16, 4, 64
SW = 1792
NT = S // 128          # 16 s-tiles
NG = NT // 4           # 4 supertiles (512 queries each)
DC = D // 128          # 9 contraction chunks
F32 = mybir.dt.float32
F32R = mybir.dt.float32r
BF16 = mybir.dt.bfloat16

_CACHED = {}


def r(ap):
    """View an AP as float32r (fast fp32 matmul mode)."""
    return ap.bitcast(F32R)


def build_graph():
    nc = bacc.Bacc()
    xT = nc.declare_dram_parameter("xT", [D, S], BF16, isOutput=False)
    wqT = nc.declare_dram_parameter("wqT", [D, 512], BF16, isOutput=False)
    wkvT = nc.declare_dram_parameter("wkvT", [D, 256], BF16, isOutput=False)
    woT = nc.declare_dram_parameter("woT", [512, D], BF16, isOutput=False)
    cosr = nc.declare_dram_parameter("cosrep", [S, 256], BF16, isOutput=False)
    sinr = nc.declare_dram_parameter("sinrep", [S, 256], BF16, isOutput=False)
    mdiag = nc.declare_dram_parameter("mdiagT", [128, 512], F32, isOutput=False)
    mtail = nc.declare_dram_parameter("mtailT", [128, 512], F32, isOutput=False)
    out_e = nc.declare_dram_parameter("out", [S, D], F32, isOutput=True)
    ck_e = nc.declare_dram_parameter("ck", [SW, 2, HD], F32, isOutput=True)
    cv_e = nc.declare_dram_parameter("cv", [SW, 2, HD], F32, isOutput=True)

    with tile.TileContext(nc) as tc:
        with (
            tc.tile_pool(name="const", bufs=1) as constp,
            tc.tile_pool(name="w", bufs=1) as wp,
            tc.tile_pool(name="persist", bufs=1) as pers,
            tc.tile_pool(name="xt", bufs=5) as xtp,
            tc.tile_pool(name="rope", bufs=3) as ropep,
            tc.tile_pool(name="ptile", bufs=18) as ppool,
            tc.tile_pool(name="outb", bufs=2) as outp,
            tc.tile_pool(name="small", bufs=2) as smallp,
            tc.tile_pool(name="attn", bufs=2) as atp,
            tc.tile_pool(name="psA", bufs=2, space="PSUM") as psA,
            tc.tile_pool(name="psB", bufs=2, space="PSUM") as psB,
            tc.tile_pool(name="psC", bufs=2, space="PSUM") as psC,
        ):
            # x tiles + per-chunk weights first, tables after
            xts = {}

            def load_xt(t):
                xt = xtp.tile([128, DC, 128], BF16, tag="xt")
                nc.sync.dma_start(
                    xt[:], xT[:, t*128:(t+1)*128].rearrange(
                        "(c p) s -> p c s", p=128))
                xts[t] = xt

            load_xt(0)
            wq_sb = wp.tile([128, DC, 512], BF16, tag="wq")
            wkv_sb = wp.tile([128, DC, 256], BF16, tag="wkv")
            for c in range(DC):
                eng = nc.scalar if c % 2 else nc.sync
                eng.dma_start(wq_sb[:, c, :], wqT[c*128:(c+1)*128, :])
                eng.dma_start(wkv_sb[:, c, :], wkvT[c*128:(c+1)*128, :])
            load_xt(1)
            load_xt(2)
            cos_sb = wp.tile([128, NT, 256], BF16, tag="cos")
            nc.scalar.dma_start(
                cos_sb[:], cosr[:].rearrange("(t p) m -> p t m", p=128))
            sin_sb = wp.tile([128, NT, 256], BF16, tag="sin")
            nc.sync.dma_start(
                sin_sb[:], sinr[:].rearrange("(t p) m -> p t m", p=128))
            md_sb = constp.tile([128, 512], F32, tag="md")
            nc.scalar.dma_start(md_sb[:], mdiag[:])
            mt_sb = constp.tile([128, 512], F32, tag="mt")
            nc.scalar.dma_start(mt_sb[:], mtail[:])
            wo_sb = wp.tile([128, 4, D], BF16, tag="wo")
            nc.gpsimd.dma_start(
                wo_sb[:], woT[:].rearrange("(c p) e -> p c e", p=128))
            ident = constp.tile([128, 128], BF16)
            make_identity(nc, ident[:])

            # persistent activations, sharded per supertile so attention
            # for gi only waits on phase-1 tiles 4gi..4gi+3 (whole-tile dep
            # granularity would otherwise serialize the phases)
            qT_q = [pers.tile([128, 4, 512], BF16, tag=f"qT{g}",
                              name=f"qT{g}") for g in range(NG)]
            kT_q = [pers.tile([128, 512], BF16, tag=f"kT{g}",
                              name=f"kT{g}") for g in range(NG)]
            vaug_q = [pers.tile([128, 4, 130], BF16, tag=f"va{g}",
                                name=f"va{g}") for g in range(NG)]
            for g in range(NG):
                nc.gpsimd.memset(vaug_q[g][:, :, 64::65], 1.0)

            # ---------------- phase 1: projections + RoPE + transposes -------
            stash = {}
            for t in range(NT):
                if t + 3 < NT:
                    load_xt(t + 3)
                xt = xts.pop(t)
                q_ps = psA.tile([128, 512], F32, tag="A")
                kv_ps = psB.tile([128, 256], F32, tag="B")
                for c in range(DC):
                    nc.tensor.matmul(q_ps[:], xt[:, c, :], wq_sb[:, c, :],
                                     start=(c == 0), stop=(c == DC-1))
                for c in range(DC):
                    nc.tensor.matmul(kv_ps[:], xt[:, c, :], wkv_sb[:, c, :],
                                     start=(c == 0), stop=(c == DC-1))

                # RoPE: q
                qro = ropep.tile([128, 512], BF16, tag="qro")
                C = cos_sb[:, t, :]
                Sn = sin_sb[:, t, :]
                ta = ropep.tile([128, 256], F32, tag="ta")
                tb = ropep.tile([128, 256], F32, tag="tb")
                a, b = q_ps[:, 0:512:2], q_ps[:, 1:512:2]
                nc.vector.tensor_mul(ta[:], a, C)
                nc.vector.tensor_mul(tb[:], b, Sn)
                nc.vector.tensor_sub(qro[:, 0:512:2], ta[:], tb[:])
                nc.vector.tensor_mul(ta[:], a, Sn)
                nc.vector.tensor_mul(tb[:], b, C)
                nc.vector.tensor_add(qro[:, 1:512:2], ta[:], tb[:])
                # RoPE: k
                kro = ropep.tile([128, 128], BF16, tag="kro")
                ka, kb = kv_ps[:, 0:128:2], kv_ps[:, 1:128:2]
                C2, S2 = cos_sb[:, t, 0:64], sin_sb[:, t, 0:64]
                ka1 = ropep.tile([128, 64], F32, tag="ka1")
                kb1 = ropep.tile([128, 64], F32, tag="kb1")
                nc.vector.tensor_mul(ka1[:], ka, C2)
                nc.vector.tensor_mul(kb1[:], kb, S2)
                nc.vector.tensor_sub(kro[:, 0:128:2], ka1[:], kb1[:])
                nc.vector.tensor_mul(ka1[:], ka, S2)
                nc.vector.tensor_mul(kb1[:], kb, C2)
                nc.vector.tensor_add(kro[:, 1:128:2], ka1[:], kb1[:])

                # v into augmented layout (ScalarE copy, psum->sbuf)
                nc.scalar.copy(
                    vaug_q[t//4][:, t % 4, :].rearrange(
                        "p (h x) -> p h x", h=2)[:, :, 0:64],
                    kv_ps[:, 128:256].rearrange("p (h x) -> p h x", h=2))

                # cache writeout for s >= 256
                if t >= 2:
                    s0 = t * 128
                    slot = s0 - SW if s0 >= SW else s0
                    nc.gpsimd.dma_start(
                        ck_e[slot:slot+128],
                        kro[:].rearrange("p (h x) -> p h x", h=2))
                    nc.gpsimd.dma_start(
                        cv_e[slot:slot+128],
                        vaug_q[t//4][:, t % 4, :].rearrange(
                            "p (h x) -> p h x", h=2)[:, :, 0:64])

                # transposes to [e, s] layout (PE transpose, bf16)
                for cc in range(4):
                    tr = psC.tile([128, 128], BF16, tag="C")
                    nc.tensor.transpose(tr[:], qro[:, cc*128:(cc+1)*128], ident[:])
                    nc.vector.tensor_copy(
                        qT_q[t//4][:, cc, (t % 4)*128:(t % 4 + 1)*128], tr[:])
                trk = psC.tile([128, 128], BF16, tag="C")
                nc.tensor.transpose(trk[:], kro[:], ident[:])
                nc.vector.tensor_copy(
                    kT_q[t//4][:, (t % 4)*128:(t % 4 + 1)*128], trk[:])

            # ---------------- phase 2: attention + WO ----------------------
            # GQA-packed: one ST matmul computes scores of ALL 4 q-heads of a
            # kv group for one (q-tile, key-block): rhs = qT[64, 4 heads, 128]
            # -> st[j, 4*128]. One PV matmul serves all 4 heads (shared V).
            for gi in range(NG):
                at_sb = atp.tile([128, 4, 512], BF16, tag="at")
                for ql in range(4):
                    qi = gi * 4 + ql
                    for h2 in range(2):
                        kjs = list(range(max(0, qi - 14), qi + 1))
                        ptiles = {}
                        # ST + masks + exp, two kj per psum tile
                        for idx in range(0, len(kjs), 2):
                            pair = kjs[idx:idx+2]
                            st = psA.tile([128, 1024], F32, tag="A")
                            p = ppool.tile([128, 1024], BF16, tag="p")
                            masked = []
                            for half, kj in enumerate(pair):
                                o = half * 512
                                nc.tensor.matmul(
                                    st[:, o:o+512],
                                    kT_q[kj//4][h2*64:(h2+1)*64,
                                                (kj % 4)*128:(kj % 4 + 1)*128],
                                    qT_q[qi//4][h2*64:h2*64+64, 0:4,
                                                (qi % 4)*128:(qi % 4 + 1)*128],
                                    start=True, stop=True)
                                if kj == qi or kj == qi - 14:
                                    masked.append((kj, o))
                                ptiles[kj] = (p, o)
                            nc.scalar.activation(
                                p[:, 0:512*len(pair)], st[:, 0:512*len(pair)],
                                mybir.ActivationFunctionType.Exp, scale=0.125)
                            # sliding-window masking: zero the invalid
                            # triangle of P on the idle GpSimd engine (the
                            # ones-row PV sums P after this, so softmax
                            # denominators stay exact)
                            for kj, o in masked:
                                sl = p[:, o:o+512].rearrange(
                                    "p (h x) -> p h x", h=4)
                                if kj == qi:
                                    # keep j <= i  (col - row >= 0)
                                    nc.gpsimd.affine_select(
                                        out=sl, in_=sl,
                                        compare_op=mybir.AluOpType.is_ge,
                                        fill=0.0, base=0,
                                        pattern=[[0, 4], [1, 128]],
                                        channel_multiplier=-1)
                                else:
                                    # tail block: keep i - j < SW (row > col)
                                    nc.gpsimd.affine_select(
                                        out=sl, in_=sl,
                                        compare_op=mybir.AluOpType.is_gt,
                                        fill=0.0, base=0,
                                        pattern=[[0, 4], [-1, 128]],
                                        channel_multiplier=1)
                        # PV accumulation over kj (shared V for the 4 heads,
                        # ones row 64 collects the softmax denominators)
                        pv = psB.tile([65, 512], F32, tag="B")
                        for kj in kjs:
                            p_, o_ = ptiles[kj]
                            nc.tensor.matmul(
                                pv[:],
                                vaug_q[kj//4][:, kj % 4, h2*65:(h2+1)*65],
                                p_[:, o_:o_+512],
                                start=(kj == kjs[0]), stop=(kj == kjs[-1]))
                        rs = smallp.tile([1, 512], F32, tag="rs")
                        nc.vector.tensor_copy(rs[:], pv[64:65, :])
                        rr = smallp.tile([1, 512], F32, tag="rr")
                        nc.vector.reciprocal_approx_fast(rr[:], rs[:])
                        Rb = smallp.tile([64, 512], F32, tag="Rb")
                        nc.gpsimd.partition_broadcast(Rb[:], rr[:])
                        nc.vector.tensor_mul(
                            at_sb[h2*64:h2*64+64, :, ql*128:(ql+1)*128],
                            pv[0:64, :].rearrange("p (h x) -> p h x", h=4),
                            Rb[:].rearrange("p (h x) -> p h x", h=4))
                # WO: out[s, d] partial for this supertile
                for ss in range(4):
                    osb = outp.tile([128, D], F32, tag="osb")
                    for nn in range(3):
                        wo_ps = psC.tile([128, 384], F32, tag="C")
                        for c in range(4):
                            nc.tensor.matmul(
                                wo_ps[:],
                                at_sb[:, c, ss*128:(ss+1)*128],
                                wo_sb[:, c, nn*384:(nn+1)*384],
                                start=(c == 0), stop=(c == 3))
                        nc.vector.tensor_copy(osb[:, nn*384:(nn+1)*384], wo_ps[:])
                    nc.sync.dma_start(
                        out_e[gi*512+ss*128:gi*512+(ss+1)*128, :], osb[:])
    nc.finalize()
    return nc


def _prep_inputs(x, wq, wk, wv, wo, freqs_cos, freqs_sin, mask):
    cosrep = np.ascontiguousarray(
        np.tile(np.asarray(freqs_cos), (1, 8))).astype(ml_dtypes.bfloat16)
    sinrep = np.ascontiguousarray(
        np.tile(np.asarray(freqs_sin), (1, 8))).astype(ml_dtypes.bfloat16)
    mdiagT = np.ascontiguousarray(
        np.tile(np.asarray(mask)[0:128, 0:128].T, (1, 4)), dtype=np.float32)
    mtailT = np.ascontiguousarray(
        np.tile(np.asarray(mask)[1792:1920, 0:128].T, (1, 4)), dtype=np.float32)
    # head-block permutation: etile c holds local heads (c, c+4) so that each
    # q head's partition half matches its kv head's half in kT
    hperm = np.array([0, 4, 1, 5, 2, 6, 3, 7])
    eperm = (hperm[:, None] * 64 + np.arange(64)[None, :]).reshape(-1)
    in_maps = []
    for b in range(B):
        for g in range(2):
            wq_g = np.asarray(wq)[512*g:512*(g+1)][eperm]      # [512, 1152]
            wo_g = np.asarray(wo)[:, 512*g:512*(g+1)][:, eperm]  # [1152, 512]
            m = {
                "xT": np.ascontiguousarray(np.asarray(x)[b].T).astype(ml_dtypes.bfloat16),
                "wqT": np.ascontiguousarray(wq_g.T).astype(ml_dtypes.bfloat16),
                "wkvT": np.ascontiguousarray(np.concatenate(
                    [np.asarray(wk)[128*g:128*(g+1)],
                     np.asarray(wv)[128*g:128*(g+1)]], 0).T).astype(
                         ml_dtypes.bfloat16),
                "woT": np.ascontiguousarray(wo_g.T).astype(ml_dtypes.bfloat16),
                "cosrep": cosrep, "sinrep": sinrep,
                "mdiagT": mdiagT, "mtailT": mtailT,
            }
            in_maps.append(m)
    return in_maps


def kernel(x, wq, wk, wv, wo, freqs_cos, freqs_sin, mask, cache_k, cache_v,
           positions, _trace=False):
    if 'nc' not in _CACHED:
        _CACHED['nc'] = build_graph()
    nc = _CACHED['nc']
    in_maps = _prep_inputs(x, wq, wk, wv, wo, freqs_cos, freqs_sin, mask)
    res = run_bass_kernel_spmd(nc, in_maps, core_ids=list(range(8)),
                               trace=_trace)
    outs = res.results
    out = np.zeros((B, S, D), np.float32)
    ck = np.zeros((B, SW, KVH, HD), np.float32)
    cv = np.zeros((B, SW, KVH, HD), np.float32)
    for b in range(B):
        for g in range(2):
            c = b*2 + g
            out[b] += outs[c]["out"]
            ck[b, :, 2*g:2*g+2] = outs[c]["ck"]
            cv[b, :, 2*g:2*g+2] = outs[c]["cv"]
    if _trace:
        return (out, ck, cv), res
    return out, ck, cv
